# revision 11
# baseline (speedup 1.0000x reference)
"""Trainium2 Bass kernel for nn_AttentionAddition (8-core SPMD).

Sharding: data-parallel over the N (RoI) dimension; Q = N/8 queries per core.

Key algebraic structure: the attention keys take only 81 distinct values.
kp[k] = relu(ce[gt[k]]) @ w_qk.T is a gather of kpre = relu(ce) @ w_qk.T
([81, D]), so E[q, k] = Ec[q, gt[k]] with Ec = exp(qp @ kpre.T / 32)
([Q, 82] incl. the dummy key). Hence
    rowsum[q] = sum_c count[c] * Ec[q, c]          (count = global histogram)
    attn @ vv = Ec[:, :81] @ Sc / rowsum,  Sc[c] = sum_{gt[k]=c} comb[k].
Sc is a [81, D] class-sum of the local comb rows, AllReduced across the 8
cores (332 KB) — replacing a 32 MB K/V AllGather — and the [N, N+1]
attention matmuls collapse to [Q, 82]-sized work.

The sem half of comb also folds: comb = OH @ (ce @ w_comb_top.T + b_comb)
+ fp @ w_comb_bot.T, with ceW precomputed ([81, D]).

Precision: fp32 PSUM everywhere; fp8(e4m3, DoubleRow 2x) for the qp/kpre/Ec
chain, comb's fp half, o1, o2 and ffn1 (weights prescaled x64 on host, the
1/64 folded into the PSUM->SBUF activation copies); bf16 for o3, ffn2 and
the PV path whose error reaches the output undiluted.

Layout: activations are kept TRANSPOSED ([feature, query], tiles
[128, chunks, q]) so every Linear maps to matmul(out, lhsT=W^T chunk,
rhs=act chunk) with no activation transposes; comb/Sc use the natural
[query, feature] layout. One final PE transpose back to [q, d].
"""

import numpy as np
import ml_dtypes

import concourse.bass as bass
import concourse.tile as tile
from concourse import bacc, mybir
from concourse.masks import make_identity

F32 = mybir.dt.float32
BF16 = mybir.dt.bfloat16
F8 = mybir.dt.float8e4
AF = mybir.ActivationFunctionType
ALU = mybir.AluOpType
DR = mybir.MatmulPerfMode.DoubleRow

NCORES = 8
N, D, S, C = 8192, 1024, 300, 80
CP1 = C + 1              # ce rows (classes + bg) = 81
KK = CP1 + 1             # + dummy key = 82
Q = N // NCORES          # queries per core = 1024
DC = D // 128            # feature chunks = 8
QC = Q // 128            # query chunks per core = 8
SCALE = 1.0 / np.sqrt(np.float32(D))  # 1/32
WS = 64.0                # fp8 weight prescale
NP_F8 = ml_dtypes.float8_e4m3


class Ctx:
    pass


def build(debug=False, single=False, repeat=1):
    nc = bacc.Bacc("TRN2", target_bir_lowering=False, debug=False,
                   num_devices=1 if single else NCORES)
    cx = Ctx()
    cx.debug = debug
    cx.single = single

    def din(name, shape, dt=F32):
        return nc.dram_tensor(name, shape, dt, kind="ExternalInput").ap()

    cx.fpT8 = din("fpT8", [D, Q], F8)          # feature_pooled shard, transposed
    cx.visT = din("visT", [D, Q], BF16)        # relu(fp).T
    cx.visT8 = din("visT8", [D, Q], F8)
    cx.ohT = din("ohT", [CP1, Q], BF16)        # one-hot(gt), class-on-partition
    cx.ohQ = din("ohQ", [Q, KK], BF16)         # one-hot(gt), query-on-partition
    cx.count = din("count", [KK, 1], BF16)     # global class histogram, [81]=1
    cx.cembT = din("cembT", [384, CP1])        # [class_embed;bg].T pad, row 300=1
    cx.wprojT = din("wprojT", [384, D])        # w_proj.T pad, row 300 = b_proj
    cx.wcombTb = din("wcombTb", [D + 1, D], BF16)  # w_comb.T rows 0..D-1 + b_comb
    cx.wcombB8 = din("wcombB8", [D, D], F8)    # w_comb.T rows D..2D-1, x64
    cx.wqkT = din("wqkT", [D, D], F8)          # w_qk.T, x64
    cx.dumT = din("dumT", [D, 1], F8)
    cx.w1T = din("w1T", [D, D // 2], F8)       # x64
    cx.b1 = din("b1", [D // 2])
    cx.w2T = din("w2T", [D, D // 2], F8)       # x64
    cx.b2 = din("b2", [D // 2])
    cx.w3T = din("w3T", [2 * D, D], BF16)
    cx.b3 = din("b3", [D])
    cx.wf1T = din("wf1T", [D, D], F8)          # x64
    cx.bf1 = din("bf1", [D])
    cx.wf2T = din("wf2T", [D, D], BF16)
    cx.bf2 = din("bf2", [D])
    cx.lng = din("lng", [D])
    cx.lnb = din("lnb", [D])

    cx.out_d = nc.dram_tensor("out", [Q, D], F32, kind="ExternalOutput").ap()

    cx.dbg = {}
    if debug:
        def dout(name, shape, dt=F32):
            cx.dbg[name] = nc.dram_tensor(name, shape, dt,
                                          kind="ExternalOutput").ap()
        dout("d_ceW", [CP1, D])
        dout("d_comb", [128, QC * D], BF16)
        dout("d_qpT", [128, DC * Q], F8)
        dout("d_EcT", [KK, Q], BF16)
        dout("d_Sc", [CP1, D])
        dout("d_rowsum", [1, Q])
        dout("d_outn", [128, DC * Q])
        dout("d_oT", [128, DC * Q])
        dout("d_normT", [128, DC * Q])

    # Sc partial-sum AllReduce buffers, one pair per d-half wave
    cx.sc_bounce = [nc.dram_tensor(f"scb{w}", [CP1, 512], F32,
                                   kind="Internal").ap() for w in range(2)]
    cx.sc_red = [nc.dram_tensor(f"scr{w}", [CP1, 512], F32,
                                kind="Internal").ap() for w in range(2)]

    with tile.TileContext(nc) as tc:
        with tc.tile_pool(name="pp", bufs=1) as pp:
            consts = pp.tile([128, 288], F32)
            cx.ident = consts[:, 0:128]
            make_identity(nc, cx.ident)
            cx.ones_c = consts[:, 128:129]
            nc.vector.memset(cx.ones_c, 1.0)
            cx.eps_t = consts[0:1, 129:130]
            nc.vector.memset(cx.eps_t, 1e-5)
            cx.ones_r = consts[0:1, 130:258]      # ones row (bias mm / bcast)
            nc.vector.memset(cx.ones_r, 1.0)
            cb = pp.tile([128, 130], BF16)
            cx.ones_cb = cb[:, 0:1]
            nc.vector.memset(cx.ones_cb, 1.0)
            cx.ones_rb = cb[0:1, 2:130]
            nc.vector.memset(cx.ones_rb, 1.0)
            c8 = pp.tile([128, 2, 128], F8)
            nc.vector.memset(c8[:], 1.0)
            cx.ones_c8p = c8                      # [128, 2, 128] fp8 ones (DR)

            for _rep in range(repeat):
                with tc.tile_pool(name="pq", bufs=1) as pq:
                    cx.pq = pq
                    _phase_a(nc, tc, cx)
                    _phase_c(nc, tc, cx)
    nc.compile()
    return nc


def _phase_a(nc, tc, cx):
    """Projections + collapsed attention.

    Order: ce/ceW + comb first so the tiny Sc AllReduce ships ASAP; the
    qp/kpre/Ec work overlaps the collective; PV + normalize close phase A.
    """
    debug, dbg = cx.debug, cx.dbg

    cx.visT_sb = visT = cx.pq.tile([128, DC, Q], BF16, name="visT_sb")
    nc.sync.dma_start(out=visT[:],
                      in_=cx.visT.rearrange("(c p) q -> p c q", p=128))
    cx.outn = outn = cx.pq.tile([128, DC, Q], F32, name="outn")

    with (
        tc.tile_pool(name="paA", bufs=1) as paA,
        tc.tile_pool(name="pap", bufs=5, space="PSUM") as pap,
    ):
        fpT8_sb = paA.tile([128, DC, Q], F8)
        nc.sync.dma_start(out=fpT8_sb[:],
                          in_=cx.fpT8.rearrange("(c p) q -> p c q", p=128))
        oh_sb = paA.tile([CP1, Q], BF16)
        nc.sync.dma_start(out=oh_sb[:], in_=cx.ohT)
        ohQ_sb = paA.tile([128, QC, KK], BF16)
        nc.sync.dma_start(out=ohQ_sb[:],
                          in_=cx.ohQ.rearrange("(c p) k -> p c k", p=128))

        # --- ce in transposed layout: ceT[d, c] ---
        cembT_sb = paA.tile([128, 3, CP1], F32)
        nc.sync.dma_start(out=cembT_sb[:],
                          in_=cx.cembT.rearrange("(c p) n -> p c n", p=128))
        wprojT_sb = paA.tile([128, 3, D], F32)
        nc.sync.dma_start(out=wprojT_sb[:],
                          in_=cx.wprojT.rearrange("(c p) d -> p c d", p=128))
        ceT_sb = paA.tile([128, DC, CP1], BF16)
        ceRT8_sb = paA.tile([128, DC, CP1], F8)
        for dc in range(DC):
            ct_ps = pap.tile([128, CP1], F32, tag="ps")
            for sc in range(3):
                nc.tensor.matmul(ct_ps[:],
                                 wprojT_sb[:, sc, dc * 128:(dc + 1) * 128],
                                 cembT_sb[:, sc, :],
                                 start=(sc == 0), stop=(sc == 2))
            nc.scalar.copy(out=ceT_sb[:, dc, :], in_=ct_ps[:])
            nc.scalar.activation(out=ceRT8_sb[:, dc, :], in_=ct_ps[:],
                                 func=AF.Relu)

        # --- ceW = (ce @ w_comb_top.T + b_comb) * WS  [81, D] bf16 ---
        ceW_sb = paA.tile([CP1, D], BF16)
        comb_sb = paA.tile([128, QC, D], BF16)
        wcb_row = paA.tile([1, D], BF16)
        nc.sync.dma_start(out=wcb_row[:], in_=cx.wcombTb[D:D + 1, :])
        wcbB8_sb = paA.tile([128, DC, D], F8)
        nc.sync.dma_start(out=wcbB8_sb[:],
                          in_=cx.wcombB8.rearrange("(c p) o -> p c o", p=128))
        with tc.tile_pool(name="paw", bufs=2) as paw:
            for ob in range(2):
                os_ = slice(ob * 512, (ob + 1) * 512)
                wcq = paw.tile([128, DC, 512], BF16, tag="wcq")
                nc.sync.dma_start(
                    out=wcq[:],
                    in_=cx.wcombTb[0:D, os_].rearrange("(c p) o -> p c o", p=128))
                cw_ps = pap.tile([CP1, 512], F32, tag="ps")
                for ic in range(DC):
                    nc.tensor.matmul(cw_ps[:], ceT_sb[:, ic, :],
                                     wcq[:, ic, :],
                                     start=(ic == 0), stop=False)
                nc.tensor.matmul(cw_ps[:], cx.ones_rb[:, 0:CP1],
                                 wcb_row[:, os_], start=False, stop=True)
                nc.scalar.activation(out=ceW_sb[:, os_], in_=cw_ps[:],
                                     func=AF.Identity, scale=WS)
        if debug:
            d_ceW = paA.tile([CP1, D], F32)
            nc.scalar.activation(out=d_ceW[:], in_=ceW_sb[:],
                                 func=AF.Identity, scale=1.0 / WS)
            nc.sync.dma_start(out=dbg["d_ceW"], in_=d_ceW[:])

        # --- comb = (OH @ ceW + fp8 @ wcbB8) / WS,  [q, d] bf16 ---
        # d-half waves: Sc's AllReduce for half 0 ships while half 1 computes
        sc_sb = paA.tile([CP1, D], F32)
        for ob in range(2):
            os_ = slice(ob * 512, (ob + 1) * 512)
            for qc in range(QC):
                cb_ps = pap.tile([128, 512], F32, tag="ps")
                nc.tensor.matmul(cb_ps[:], oh_sb[:, qc * 128:(qc + 1) * 128],
                                 ceW_sb[:, os_], start=True, stop=False)
                for ic in range(0, DC, 2):
                    nc.tensor.matmul(cb_ps[:],
                                     fpT8_sb[:, ic:ic + 2,
                                             qc * 128:(qc + 1) * 128],
                                     wcbB8_sb[:, ic:ic + 2, os_],
                                     start=False, stop=(ic == DC - 2),
                                     perf_mode=DR)
                nc.scalar.activation(out=comb_sb[:, qc, os_], in_=cb_ps[:],
                                     func=AF.Identity, scale=1.0 / WS)
            # Sc partial for this d-half = OH^T @ comb[:, :, half]
            sc_ps = pap.tile([CP1, 512], F32, tag="ps")
            for qc in range(QC):
                nc.tensor.matmul(sc_ps[:], ohQ_sb[:, qc, 0:CP1],
                                 comb_sb[:, qc, os_],
                                 start=(qc == 0), stop=(qc == QC - 1))
            nc.scalar.copy(out=sc_sb[:, os_], in_=sc_ps[:])
            nc.sync.dma_start(out=cx.sc_bounce[ob], in_=sc_sb[:, os_])
            if cx.single:
                nc.sync.dma_start(out=cx.sc_red[ob], in_=cx.sc_bounce[ob])
            else:
                nc.gpsimd.collective_compute(
                    "AllReduce", ALU.add,
                    replica_groups=[list(range(NCORES))],
                    ins=[cx.sc_bounce[ob]], outs=[cx.sc_red[ob]])
        if debug:
            nc.sync.dma_start(out=dbg["d_comb"],
                              in_=comb_sb[:].rearrange("p c q -> p (c q)"))

        # --- qp = wqk @ visT (fp8 DR); kpreT = wqk @ ceR^T (+dummy col) ---
        visT8_sb = paA.tile([128, DC, Q], F8)
        nc.sync.dma_start(out=visT8_sb[:],
                          in_=cx.visT8.rearrange("(c p) q -> p c q", p=128))
        qpT8 = paA.tile([128, DC, Q], F8)
        kpreT8 = paA.tile([128, DC, 128], F8)   # cols KK..127 zero-padded (DR
        nc.vector.memset(kpreT8[:, :, KK:128], 0.0)  # lhsT plane must be 128)
        dum_sb = paA.tile([128, DC, 1], F8)
        nc.sync.dma_start(out=dum_sb[:],
                          in_=cx.dumT.rearrange("(c p) o -> p c o", p=128))
        wqk_sb = paA.tile([128, DC, D], F8)
        nc.sync.dma_start(out=wqk_sb[:],
                          in_=cx.wqkT.rearrange("(c p) o -> p c o", p=128))
        for oc in range(DC):
            for qh in range(2):
                qs = slice(qh * 512, (qh + 1) * 512)
                qk_ps = pap.tile([128, 512], F32, tag="ps")
                for ic in range(0, DC, 2):
                    nc.tensor.matmul(qk_ps[:],
                                     wqk_sb[:, ic:ic + 2,
                                            oc * 128:(oc + 1) * 128],
                                     visT8_sb[:, ic:ic + 2, qs],
                                     start=(ic == 0), stop=(ic == DC - 2),
                                     perf_mode=DR)
                nc.scalar.activation(out=qpT8[:, oc, qs], in_=qk_ps[:],
                                     func=AF.Identity, scale=1.0 / WS)
            kp_ps = pap.tile([128, CP1], F32, tag="ps")
            for ic in range(0, DC, 2):
                nc.tensor.matmul(kp_ps[:],
                                 wqk_sb[:, ic:ic + 2, oc * 128:(oc + 1) * 128],
                                 ceRT8_sb[:, ic:ic + 2, :],
                                 start=(ic == 0), stop=(ic == DC - 2),
                                 perf_mode=DR)
            nc.scalar.activation(out=kpreT8[:, oc, 0:CP1], in_=kp_ps[:],
                                 func=AF.Identity, scale=1.0 / WS)
            nc.scalar.copy(out=kpreT8[:, oc, CP1:KK], in_=dum_sb[:, oc, :])
        if debug:
            nc.sync.dma_start(out=dbg["d_qpT"],
                              in_=qpT8[:].rearrange("p c q -> p (c q)"))

        # --- Ec^T = exp(kpre @ qp^T / 32)  [82, Q] bf16 ---
        EcTb = paA.tile([KK, Q], BF16)
        for qh in range(2):
            qs = slice(qh * 512, (qh + 1) * 512)
            ec_ps = pap.tile([128, 512], F32, tag="ps")
            for dc in range(0, DC, 2):
                nc.tensor.matmul(ec_ps[:], kpreT8[:, dc:dc + 2, :],
                                 qpT8[:, dc:dc + 2, qs],
                                 start=(dc == 0), stop=(dc == DC - 2),
                                 perf_mode=DR)
            nc.scalar.activation(out=EcTb[:, qs], in_=ec_ps[0:KK, :],
                                 func=AF.Exp, scale=float(SCALE))
        if debug:
            nc.sync.dma_start(out=dbg["d_EcT"], in_=EcTb[:])

        # --- rowsum = count . Ec; recip broadcast to 128 partitions ---
        count_sb = paA.tile([KK, 1], BF16)
        nc.sync.dma_start(out=count_sb[:], in_=cx.count)
        rowsum = paA.tile([1, Q], F32)
        recip = paA.tile([1, Q], F32)
        recipb = paA.tile([128, Q], F32)
        for qh in range(2):
            qs = slice(qh * 512, (qh + 1) * 512)
            rs_ps = pap.tile([1, 512], F32, tag="ps")
            nc.tensor.matmul(rs_ps[:], count_sb[:], EcTb[:, qs],
                             start=True, stop=True)
            nc.vector.tensor_copy(rowsum[:, qs], rs_ps[:])
        nc.vector.reciprocal(recip[:], rowsum[:])
        for qh in range(2):
            qs = slice(qh * 512, (qh + 1) * 512)
            b_ps = pap.tile([128, 512], F32, tag="ps")
            nc.tensor.matmul(b_ps[:], cx.ones_r, recip[:, qs],
                             start=True, stop=True)
            nc.scalar.copy(out=recipb[:, qs], in_=b_ps[:])
        if debug:
            nc.sync.dma_start(out=dbg["d_rowsum"], in_=rowsum[:])

        # --- PV: out^T = Sc^T-contract Ec^T, normalized by rowsum ---
        scg_sb = paA.tile([CP1, D], F32)
        scgb = paA.tile([CP1, D], BF16)
        for w in range(2):
            ws_ = slice(w * 512, (w + 1) * 512)
            nc.sync.dma_start(out=scg_sb[:, ws_], in_=cx.sc_red[w])
            nc.scalar.copy(out=scgb[:, ws_], in_=scg_sb[:, ws_])
        for dc in range(DC):
            for qh in range(2):
                qs = slice(qh * 512, (qh + 1) * 512)
                o_ps = pap.tile([128, 512], F32, tag="ps")
                nc.tensor.matmul(o_ps[:],
                                 scgb[:, dc * 128:(dc + 1) * 128],
                                 EcTb[0:CP1, qs], start=True, stop=True)
                nc.vector.tensor_mul(outn[:, dc, qs], o_ps[:], recipb[:, qs])
        if debug:
            nc.sync.dma_start(out=dbg["d_Sc"], in_=scg_sb[:])
            nc.sync.dma_start(out=dbg["d_outn"],
                              in_=outn[:].rearrange("p c q -> p (c q)"))


def _phase_c(nc, tc, cx):
    """Epilogue: o1/o2/o3, LayerNorm, FFN, final relu-add, transpose, store."""
    debug, dbg = cx.debug, cx.dbg
    outn = cx.outn
    with (
        tc.tile_pool(name="pcB", bufs=1) as pcB,
        tc.tile_pool(name="pcp", bufs=8, space="PSUM") as pcp,
    ):
        # all per-feature bias vectors packed into one 4KB tile
        bias = pcB.tile([128, 48], F32)
        b1_sb = bias[:, 0:4]
        nc.sync.dma_start(out=b1_sb, in_=cx.b1.rearrange("(c p) -> p c", p=128))
        b2_sb = bias[:, 4:8]
        nc.sync.dma_start(out=b2_sb, in_=cx.b2.rearrange("(c p) -> p c", p=128))
        b3_sb = bias[:, 8:16]
        nc.sync.dma_start(out=b3_sb, in_=cx.b3.rearrange("(c p) -> p c", p=128))
        bf1_sb = bias[:, 16:24]
        nc.sync.dma_start(out=bf1_sb, in_=cx.bf1.rearrange("(c p) -> p c", p=128))
        bf2_sb = bias[:, 24:32]
        nc.sync.dma_start(out=bf2_sb, in_=cx.bf2.rearrange("(c p) -> p c", p=128))
        lnb2_sb = bias[:, 32:40]                 # ln_b + bf2 folded
        nc.sync.dma_start(out=lnb2_sb, in_=cx.lnb.rearrange("(c p) -> p c", p=128))
        nc.vector.tensor_add(lnb2_sb, lnb2_sb, bf2_sb)
        lng_sb = bias[:, 40:48]
        nc.sync.dma_start(out=lng_sb, in_=cx.lng.rearrange("(c p) -> p c", p=128))

        with tc.tile_pool(name="pcOT", bufs=1) as pcOT:
            oT8_sb = pcOT.tile([128, DC, Q], F8)
            oT32 = pcOT.tile([128, DC, Q], F32)   # fp32 copy for the LN path
            cx._oT32 = oT32

            with tc.tile_pool(name="pcA", bufs=1) as pcA:
                vis2 = cx.visT_sb
                o1_sb = pcA.tile([128, 4, Q], BF16)
                o2_sb = pcA.tile([128, 4, Q], BF16)
                with tc.tile_pool(name="pcZ", bufs=1) as pcZ:
                    w1_sb = pcZ.tile([128, DC, 512], F8)
                    nc.sync.dma_start(out=w1_sb[:],
                                      in_=cx.w1T.rearrange("(c p) o -> p c o", p=128))
                    w2_sb = pcZ.tile([128, DC, 512], F8)
                    nc.sync.dma_start(out=w2_sb[:],
                                      in_=cx.w2T.rearrange("(c p) o -> p c o", p=128))
                    for half, (o_sb, wh_sb, bh_sb) in enumerate(
                            [(o1_sb, w1_sb, b1_sb), (o2_sb, w2_sb, b2_sb)]):
                        for qh in range(Q // 512):
                            qs = slice(qh * 512, (qh + 1) * 512)
                            z_sb = pcZ.tile([128, DC, 512], F8, tag="z", bufs=1)
                            for dc in range(DC):
                                if half == 0:
                                    nc.vector.tensor_mul(z_sb[:, dc, :],
                                                         outn[:, dc, qs],
                                                         vis2[:, dc, qs])
                                else:
                                    nc.vector.tensor_sub(z_sb[:, dc, :],
                                                         vis2[:, dc, qs],
                                                         outn[:, dc, qs])
                            for oc in range(4):
                                m_ps = pcp.tile([128, 512], F32, tag="cps")
                                for ic in range(0, DC, 2):
                                    nc.tensor.matmul(
                                        m_ps[:],
                                        wh_sb[:, ic:ic + 2,
                                              oc * 128:(oc + 1) * 128],
                                        z_sb[:, ic:ic + 2, :],
                                        start=(ic == 0), stop=(ic == DC - 2),
                                        perf_mode=DR)
                                nc.scalar.activation(out=o_sb[:, oc, qs],
                                                     in_=m_ps[:], func=AF.Relu,
                                                     scale=1.0 / WS,
                                                     bias=bh_sb[:, oc:oc + 1])

                # o = w3 @ [o1; o2; vis] + b3  (transposed out [d, q])
                with tc.tile_pool(name="pcW", bufs=2) as pcW:
                    for oc in range(DC):
                        w3c = pcW.tile([128, 16, 128], BF16, tag="w3c")
                        nc.sync.dma_start(
                            out=w3c[:],
                            in_=cx.w3T[:, oc * 128:(oc + 1) * 128]
                            .rearrange("(c p) o -> p c o", p=128))
                        for qh in range(Q // 512):
                            qs = slice(qh * 512, (qh + 1) * 512)
                            m_ps = pcp.tile([128, 512], F32, tag="cps")
                            for ic in range(16):
                                rhs = (o1_sb[:, ic, qs] if ic < 4 else
                                       o2_sb[:, ic - 4, qs] if ic < 8 else
                                       vis2[:, ic - 8, qs])
                                nc.tensor.matmul(m_ps[:], w3c[:, ic, :], rhs,
                                                 start=(ic == 0), stop=(ic == 15))
                            nc.scalar.activation(out=oT8_sb[:, oc, qs],
                                                 in_=m_ps[:],
                                                 func=AF.Identity,
                                                 bias=b3_sb[:, oc:oc + 1])
                            nc.scalar.activation(out=oT32[:, oc, qs], in_=m_ps[:],
                                                 func=AF.Identity,
                                                 bias=b3_sb[:, oc:oc + 1])
            if debug:
                nc.sync.dma_start(out=dbg["d_oT"],
                                  in_=oT32[:].rearrange("p c q -> p (c q)"))

            with tc.tile_pool(name="pcN", bufs=1) as pcN:
                # LayerNorm over feature dim (partition reduce via ones-matmul)
                normT = pcN.tile([128, DC, Q], F32)
                with tc.tile_pool(name="pcL", bufs=2) as pcL:
                    for qh in range(Q // 512):
                        qs = slice(qh * 512, (qh + 1) * 512)
                        sum_ps = pcp.tile([128, 512], F32, tag="cps")
                        ssq_ps = pcp.tile([128, 512], F32, tag="cps")
                        for dc in range(0, DC, 2):
                            nc.tensor.matmul(sum_ps[:], cx.ones_c8p[:],
                                             oT8_sb[:, dc:dc + 2, qs],
                                             start=(dc == 0), stop=(dc == DC - 2),
                                             perf_mode=DR)
                            sq_t = pcL.tile([128, 2, 512], F8, tag="sq")
                            nc.scalar.activation(out=sq_t[:, 0, :],
                                                 in_=oT8_sb[:, dc, qs],
                                                 func=AF.Square)
                            nc.scalar.activation(out=sq_t[:, 1, :],
                                                 in_=oT8_sb[:, dc + 1, qs],
                                                 func=AF.Square)
                            nc.tensor.matmul(ssq_ps[:], cx.ones_c8p[:],
                                             sq_t[:],
                                             start=(dc == 0), stop=(dc == DC - 2),
                                             perf_mode=DR)
                        st = pcL.tile([1, 3, 512], F32, tag="st", bufs=1)
                        slot_a, slot_b, slot_c = (st[:, i, :] for i in range(3))
                        nc.scalar.mul(out=slot_a, in_=sum_ps[0:1, :], mul=1.0 / D)  # mu
                        nc.scalar.mul(out=slot_b, in_=ssq_ps[0:1, :], mul=1.0 / D)  # E[x^2]
                        nc.vector.tensor_mul(slot_c, slot_a, slot_a)    # mu^2
                        nc.vector.tensor_sub(slot_b, slot_b, slot_c)    # var
                        nc.scalar.activation(out=slot_b, in_=slot_b, func=AF.Sqrt,
                                             bias=cx.eps_t)             # sd
                        nc.vector.reciprocal(slot_c, slot_b)            # c1 = rstd
                        nc.vector.tensor_mul(slot_a, slot_a, slot_c)    # c0 = mu*rstd
                        c1b = pcL.tile([128, 512], F32, tag="c1b")
                        c0b = pcL.tile([128, 512], F32, tag="c0b")
                        for src, dst in [(slot_c, c1b), (slot_a, c0b)]:
                            bb_ps = pcp.tile([128, 512], F32, tag="cps")
                            nc.tensor.matmul(bb_ps[:], cx.ones_r, src,
                                             start=True, stop=True)
                            nc.scalar.copy(out=dst[:], in_=bb_ps[:])
                        for dc in range(DC):
                            tmp = pcL.tile([128, 512], F32, tag="lnt")
                            nc.vector.tensor_mul(tmp[:], oT32[:, dc, qs], c1b[:])
                            nc.vector.tensor_sub(tmp[:], tmp[:], c0b[:])
                            nc.vector.tensor_scalar(
                                out=normT[:, dc, qs], in0=tmp[:],
                                scalar1=lng_sb[:, dc:dc + 1],
                                scalar2=lnb2_sb[:, dc:dc + 1],
                                op0=ALU.mult, op1=ALU.add)
                if debug:
                    nc.sync.dma_start(out=dbg["d_normT"],
                                      in_=normT[:].rearrange("p c q -> p (c q)"))

                # FFN layer 1 (fp8 DR, consumes oT8), weights streamed
                f1_sb = pcN.tile([128, DC, Q], BF16)
                with tc.tile_pool(name="pcM1", bufs=2) as pcM1:
                    for oc in range(DC):
                        wf1c = pcM1.tile([128, DC, 128], F8, tag="wf1c")
                        nc.sync.dma_start(
                            out=wf1c[:],
                            in_=cx.wf1T[:, oc * 128:(oc + 1) * 128]
                            .rearrange("(c p) o -> p c o", p=128))
                        for qh in range(Q // 512):
                            qs = slice(qh * 512, (qh + 1) * 512)
                            m_ps = pcp.tile([128, 512], F32, tag="cps")
                            for ic in range(0, DC, 2):
                                nc.tensor.matmul(m_ps[:],
                                                 wf1c[:, ic:ic + 2, :],
                                                 oT8_sb[:, ic:ic + 2, qs],
                                                 start=(ic == 0),
                                                 stop=(ic == DC - 2),
                                                 perf_mode=DR)
                            nc.scalar.activation(out=f1_sb[:, oc, qs], in_=m_ps[:],
                                                 func=AF.Relu, scale=1.0 / WS,
                                                 bias=bf1_sb[:, oc:oc + 1])

                # FFN layer 2 + LayerNorm residual + final relu
                with tc.tile_pool(name="pcM2", bufs=1) as pcM2:
                    fin_sb = pcM2.tile([128, DC, Q], F32)
                    with tc.tile_pool(name="pcM2w", bufs=2) as pcM2w:
                        for oc in range(DC):
                            wf2c = pcM2w.tile([128, DC, 128], BF16, tag="wf2c")
                            nc.sync.dma_start(
                                out=wf2c[:],
                                in_=cx.wf2T[:, oc * 128:(oc + 1) * 128]
                                .rearrange("(c p) o -> p c o", p=128))
                            for qh in range(Q // 512):
                                qs = slice(qh * 512, (qh + 1) * 512)
                                m_ps = pcp.tile([128, 512], F32, tag="cps")
                                for ic in range(DC):
                                    nc.tensor.matmul(m_ps[:], wf2c[:, ic, :],
                                                     f1_sb[:, ic, qs],
                                                     start=(ic == 0),
                                                     stop=(ic == DC - 1))
                                ts = pcM2w.tile([128, 512], F32, tag="ts")
                                nc.vector.tensor_add(ts[:], m_ps[:],
                                                     normT[:, oc, qs])
                                nc.scalar.activation(out=fin_sb[:, oc, qs],
                                                     in_=ts[:], func=AF.Relu)

                    # transpose [d, q] -> [q, d] on the PE
                    # (oT32 is dead after the LN stage; reuse its space)
                    onat = cx._oT32
                    for dc in range(DC):
                        for qc in range(QC):
                            t_ps = pcp.tile([128, 128], F32, tag="cps")
                            nc.tensor.transpose(
                                t_ps[:], fin_sb[:, dc, qc * 128:(qc + 1) * 128],
                                cx.ident)
                            nc.scalar.copy(
                                out=onat[:, qc, dc * 128:(dc + 1) * 128],
                                in_=t_ps[:])
                    nc.sync.dma_start(
                        out=cx.out_d.rearrange("(c p) d -> p c d", p=128),
                        in_=onat[:])


# ---------------------------------------------------------------------------
# Host side
# ---------------------------------------------------------------------------

_CACHE = {}


def _f8(x, scale=1.0):
    y = np.asarray(x, np.float32) * scale
    np.clip(y, -240.0, 240.0, out=y)
    return y.astype(NP_F8)


def _prep_in_maps(inputs):
    f32 = np.float32
    fp = np.asarray(inputs["feature_pooled"], f32)
    gt = np.asarray(inputs["gt_classes"]).astype(np.int64)
    ce = np.asarray(inputs["class_embed"], f32)
    bg = np.asarray(inputs["bg_embed"], f32)
    w_proj = np.asarray(inputs["w_proj"], f32)
    b_proj = np.asarray(inputs["b_proj"], f32)
    w_comb = np.asarray(inputs["w_comb"], f32)
    b_comb = np.asarray(inputs["b_comb"], f32)
    w_qk = np.asarray(inputs["w_qk"], f32)
    dummy = np.asarray(inputs["dummy"], f32)

    cembT = np.zeros((384, CP1), f32)
    cembT[:S] = np.concatenate([ce, bg], 0).T
    cembT[S] = 1.0
    wprojT = np.zeros((384, D), f32)
    wprojT[:S] = w_proj.T
    wprojT[S] = b_proj
    wcombT = np.ascontiguousarray(w_comb.T)
    wcombTb = np.concatenate([wcombT[:D], b_comb[None, :]], 0)

    count = np.zeros((KK, 1), f32)
    count[0:CP1, 0] = np.bincount(gt, minlength=CP1)
    count[CP1, 0] = 1.0   # dummy key

    shared = {
        "count": count.astype(ml_dtypes.bfloat16),
        "cembT": cembT,
        "wprojT": wprojT,
        "wcombTb": wcombTb.astype(ml_dtypes.bfloat16),
        "wcombB8": _f8(wcombT[D:], WS),
        "wqkT": _f8(w_qk.T, WS),
        "dumT": _f8(dummy.T),
        "w1T": _f8(np.asarray(inputs["w1"], f32).T, WS),
        "b1": np.asarray(inputs["b1"], f32),
        "w2T": _f8(np.asarray(inputs["w2"], f32).T, WS),
        "b2": np.asarray(inputs["b2"], f32),
        "w3T": np.ascontiguousarray(np.asarray(inputs["w3"], f32).T).astype(ml_dtypes.bfloat16),
        "b3": np.asarray(inputs["b3"], f32),
        "wf1T": _f8(np.asarray(inputs["wf1"], f32).T, WS),
        "bf1": np.asarray(inputs["bf1"], f32),
        "wf2T": np.ascontiguousarray(np.asarray(inputs["wf2"], f32).T).astype(ml_dtypes.bfloat16),
        "bf2": np.asarray(inputs["bf2"], f32),
        "lng": np.asarray(inputs["ln_g"], f32),
        "lnb": np.asarray(inputs["ln_b"], f32),
    }
    in_maps = []
    for c in range(NCORES):
        qs = slice(c * Q, (c + 1) * Q)
        gtl = gt[qs]
        oh = np.zeros((CP1, Q), ml_dtypes.bfloat16)
        oh[gtl, np.arange(Q)] = 1.0
        ohq = np.zeros((Q, KK), ml_dtypes.bfloat16)
        ohq[np.arange(Q), gtl] = 1.0
        fpTl = np.ascontiguousarray(fp[qs].T)
        visTl = np.maximum(fpTl, 0)
        m = dict(shared)
        m["fpT8"] = _f8(fpTl)
        m["visT"] = visTl.astype(ml_dtypes.bfloat16)
        m["visT8"] = _f8(visTl)
        m["ohT"] = oh
        m["ohQ"] = ohq
        in_maps.append(m)
    return in_maps


def get_nc(debug=False):
    key = ("nc", debug)
    if key not in _CACHE:
        _CACHE[key] = build(debug=debug)
    return _CACHE[key]


def kernel(**inputs):
    from concourse import bass_utils
    try:
        # persistent XLA/PJRT compile cache so repeat invocations (fresh
        # processes included) skip the NEFF compile
        import jax
        jax.config.update("jax_compilation_cache_dir", "/tmp/jax_neff_cache")
        jax.config.update("jax_persistent_cache_min_compile_time_secs", 1.0)
        jax.config.update("jax_persistent_cache_min_entry_size_bytes", 0)
    except Exception:
        pass
    nc = get_nc(debug=False)
    in_maps = _prep_in_maps(inputs)
    res = bass_utils.run_bass_kernel_spmd(
        nc, in_maps, core_ids=list(range(NCORES)), trace=False)
    return np.concatenate([res.results[c]["out"] for c in range(NCORES)], axis=0)
